# revision 63
# baseline (speedup 1.0000x reference)
"""Trainium2 Bass kernel for an AttentionBlock:
GroupNorm(8 groups) -> q/k/v dense -> softmax(q k^T / sqrt(d)) v -> proj -> +residual(xn).

Sharding: 8 cores = (batch b in 0..3) x (half h in 0..1). Core (b, h) receives
x[b] in both layouts ([C, T] for q/k, [T, C] for attn@x) with its half of the
T=4096 tokens rolled to the front; it computes group norm + k for all tokens
and attention / projection / residual for its own 2048 query rows. Output is
produced transposed ([C, TM]); the host transposes back while gathering.

v elimination: attn_n@V@Wp == (attn_n@x) @ (diag(A_gn) Wv Wp) + 1⊗cvec
(rowsums of normalized attn are 1), so the v dense never runs: the po matmul
consumes fp8 x directly (natural [s, c] layout with a ones column for the
softmax denominator) and the projection uses the on-device fused weight
Wvp = (A_gn ⊙rows Wv) @ Wp.  cvec = (B_gn@Wv + bv)@Wp + bp is the same fc2
constant as the classic path.

Numerics: groupnorm+residual fp32; q/k/scores/exp/po in fp8-e4m3 DoubleRow
(q/k weights carry 16x for fp8 range, undone in the exp scale; exp has a
-ln(32) shift); projection bf16.

Schedule: ACT does ONLY a few tableless Copy casts, the sqrt, and the 64-exp
softmax stream (all other psum evictions live on DVE/GpSimd), the PE is
HAM-warmed with real bf16 matmuls (transposes don't count as PE-busy), x
streams in over four DMA rings, and all of q/k is produced before the stream
so the exp run is gapless.
"""

import numpy as np
from contextlib import ExitStack

import concourse.bass as bass
import concourse.tile as tile
from concourse import mybir
from concourse.bass import ts
from concourse.masks import make_identity
from concourse.bass_utils import run_bass_kernel_spmd

F32 = mybir.dt.float32
BF16 = mybir.dt.bfloat16
FP8 = mybir.dt.float8e4
AF = mybir.ActivationFunctionType
ALU = mybir.AluOpType
DR = mybir.MatmulPerfMode.DoubleRow

N_CORES = 8
GROUPS = 8
EPS = 1e-3
P = 128
LN4 = 3.4657359027997265  # ln(32): softmax-invariant shift; keeps exp < 240
N_WARM = 76               # f32 N=128 warmup matmuls (HAM + queue pacing)


def build_nc(T=4096, C=256):
    TM = T // 2          # rows (queries) this core owns
    CT = C // P          # channel tiles (2)
    NS = T // P          # key tiles (32)
    Tc = 512             # query chunk
    NT = TM // Tc        # t-chunks of the query rows (4)
    JT = Tc // P         # 128-row output subtiles per t-chunk (4)
    NPAIR = NS // 2      # score pairs (1024-wide exp groups) per t-chunk (16)
    GS = C // GROUPS     # channels per group (32)
    GPT = P // GS        # groups per channel tile (4)
    NB = T // 512        # x chunks per channel tile (8)
    VC = 272             # x8n row stride (C + den col + pad to 16B)
    sc16 = float(C) ** -0.5 / 256.0

    assert CT == 2 and TM % Tc == 0 and T % 512 == 0

    nc = bass.Bass()

    xT_d = nc.dram_tensor("xT", [C, T], BF16, kind="ExternalInput")
    xT8_d = nc.dram_tensor("xT8", [C, T], FP8, kind="ExternalInput")
    xN8_d = nc.dram_tensor("xN8", [P, (T // P) * VC], FP8, kind="ExternalInput")
    gamma_d = nc.dram_tensor("gamma", [C], F32, kind="ExternalInput")
    beta_d = nc.dram_tensor("beta", [C], F32, kind="ExternalInput")
    Wq_d = nc.dram_tensor("Wq", [C, C], F32, kind="ExternalInput")
    Wk_d = nc.dram_tensor("Wk", [C, C], F32, kind="ExternalInput")
    Wv_d = nc.dram_tensor("Wv", [C, C], F32, kind="ExternalInput")
    Wp_d = nc.dram_tensor("Wp", [C, C], F32, kind="ExternalInput")
    bq_d = nc.dram_tensor("bq", [C], F32, kind="ExternalInput")
    bk_d = nc.dram_tensor("bk", [C], F32, kind="ExternalInput")
    bv_d = nc.dram_tensor("bv", [C], F32, kind="ExternalInput")
    bp_d = nc.dram_tensor("bp", [C], F32, kind="ExternalInput")
    gind_d = nc.dram_tensor("gind", [P, GPT], F32, kind="ExternalInput")
    gindT_d = nc.dram_tensor("gindT", [GPT, P], F32, kind="ExternalInput")
    out_d = nc.dram_tensor("outT", [C, TM], F32, kind="ExternalOutput")

    with ExitStack() as ctx:
        tc = ctx.enter_context(tile.TileContext(nc))

        const = ctx.enter_context(tc.tile_pool(name="const", bufs=1))
        persist = ctx.enter_context(tc.tile_pool(name="persist", bufs=1))
        # PSUM: acc tag = 1-bank slots x4; big tag = 2-bank slots x2 (8 banks)
        ps_acc = ctx.enter_context(tc.tile_pool(name="ps_acc", bufs=4, space="PSUM"))
        ps_big = ctx.enter_context(tc.tile_pool(name="ps_big", bufs=2, space="PSUM"))

        # ---- identity + warmup FIRST: the iota must precede the DMA
        # descriptor instructions on the gpsimd queue, or the warm matmuls
        # (and psg behind them on the PE FIFO) slip by ~7us ----
        ident = const.tile([P, P], F32, tag="ident")
        make_identity(nc, ident)
        # Real matmuls (NOT transposes - those don't count as PE-busy for
        # HAM) keep the PE warming from t~7us until the gn-stat matmuls.
        # They must depend ONLY on make_identity: any DVE-produced operand
        # gets scheduled behind the bn_stats queue.
        warm = ps_acc.tile([P, P], F32, tag="acc", name="warm")
        for _ in range(N_WARM):
            nc.tensor.matmul(warm, ident, ident, start=True, stop=True)
        ident_bf = const.tile([P, P], BF16, tag="identb")
        nc.vector.tensor_copy(ident_bf, ident)

        # ---- x loads: three rings, phase-ordered per ring so the global
        # arrival order is x-bf16 -> Wk/Wq -> xT8 halves -> Wv/Wp + xN8
        # quarters (each gated only by how soon its consumer runs) ----
        xin = ctx.enter_context(tc.tile_pool(name="xin", bufs=1))
        gnst = ctx.enter_context(tc.tile_pool(name="gnst", bufs=2))
        # x-bf16 as four 512KB transfers (big transfers spread across all
        # 16 SDMA slots); the h0 halves lead on both rings so the ib-major
        # bn_stats stream starts as soon as both h0s land
        xT_sb = []
        stats = []
        for ct in range(CT):
            xt = xin.tile([P, T], BF16, tag=f"x{ct}", name=f"x{ct}")
            st = gnst.tile([P, NB, 6], F32, tag=f"bn{ct}", name=f"bn{ct}")
            xT_sb.append(xt)
            stats.append(st)
        # ct-alternating so BOTH channel tiles of each token range land
        # together (bn_stats consumes ib-major across cts)
        for cb in range(4):
            for k2, eng in ((0, nc.sync), (1, nc.gpsimd)):
                ct = (cb + k2) % 2
                eng.dma_start(
                    xT_sb[ct][:, ts(cb, 1024)],
                    xT_d[ts(ct, P), ts(cb, 1024)],
                )

        # ACT table preloads: Sqrt now; Exp is loaded right after the real
        # SQRT so the set resident during the stream is Exp, no mid-switch.
        eps_sb = const.tile([P, 1], F32, tag="eps")
        nc.vector.memset(eps_sb, EPS)
        nl4_sb = const.tile([P, 1], F32, tag="nl4")
        nc.vector.memset(nl4_sb, -LN4)
        scratch1 = const.tile([P, 1], F32, tag="scr1")
        nc.scalar.activation(scratch1, eps_sb, AF.Sqrt, bias=eps_sb)

        # ---- constants / small parameter loads (scalar ring tail) ----
        gind_sb = const.tile([P, GPT], F32, tag="gind")
        nc.scalar.dma_start(gind_sb, gind_d[:, :])
        gindT_sb = const.tile([GPT, P], F32, tag="gindT")
        nc.scalar.dma_start(gindT_sb, gindT_d[:, :])

        def col2(dram_vec, tag):
            t = const.tile([P, CT], F32, tag=tag, name=tag)
            nc.scalar.dma_start(t, dram_vec.rearrange("(c p) -> p c", p=P))
            return t

        gamma2 = col2(gamma_d, "gamma2")
        beta2 = col2(beta_d, "beta2")
        bq_c = col2(bq_d, "bqc")
        bk_c = col2(bk_d, "bkc")
        bv_c = col2(bv_d, "bvc")
        bp_c = col2(bp_d, "bpc")

        # weight raw staging behind each ring's x-bf16 chunks: Wk on sync,
        # Wq on gpsimd (both gate the gn->k chain), Wv/Wp on scalar (late
        # consumers)
        wraw = ctx.enter_context(tc.tile_pool(name="wraw", bufs=8))

        def w_raw_tiles(dram_w, tag, eng):
            tiles = []
            for ci in range(CT):
                raw = wraw.tile([P, C], F32, tag="wraw", name=f"{tag}{ci}raw")
                eng.dma_start(raw, dram_w[ts(ci, P), :])
                tiles.append(raw)
            return tiles

        Wk_raw = w_raw_tiles(Wk_d, "wk", nc.sync)
        Wq_raw = w_raw_tiles(Wq_d, "wq", nc.gpsimd)
        Wv_raw = w_raw_tiles(Wv_d, "wv", nc.scalar)
        Wp_raw = w_raw_tiles(Wp_d, "wp", nc.scalar)
        # (smalls + Wv/Wp on the scalar ring total ~0.57MB - they trickle
        # alongside the x loads without moving the critical arrivals much)

        # fp8 copies of x come PRE-CAST from the host (xT8 [C,T]; xN8 packed
        # [P, NS*VC] natural-layout with the ones column baked in) - both
        # fully contiguous per partition, issued AFTER the weights so the
        # critical loads win the SDMA round-robin.
        x8 = persist.tile([P, CT, T], FP8, tag="x8")
        x8n = persist.tile([P, NS, VC], FP8, tag="x8n")
        for hh, eng in ((0, nc.sync), (1, nc.gpsimd)):
            eng.dma_start(
                x8[:, :, ts(hh, T // 2)],
                xT8_d[:, ts(hh, T // 2)].rearrange("(a p) t -> p a t", p=P),
            )
        x8n_flat = x8n.rearrange("p a b -> p (a b)")
        for qq, eng in ((0, nc.sync), (1, nc.gpsimd), (2, nc.sync), (3, nc.gpsimd)):
            eng.dma_start(
                x8n_flat[:, ts(qq, 8 * VC)],
                xN8_d[:, ts(qq, 8 * VC)],
            )

        # ---- fp8 casts of x^T (a few tableless Copy casts on ACT, rest on
        # DVE/GpSimd) + bn_stats on DVE, streaming behind the x DMAs ----
        for ib in range(NB):
            for ct in range(CT):
                nc.vector.bn_stats(stats[ct][:, ib, :], xT_sb[ct][:, ts(ib, 512)])

        # ---- group norm stats -> per-channel A (scale), B (shift) ----
        mv2 = gnst.tile([P, CT, 2], F32, tag="mv2")
        for ct in range(CT):
            nc.vector.bn_aggr(mv2[:, ct, :], stats[ct])
        rhs2 = gnst.tile([P, CT, 2], F32, tag="rhs2")
        nc.vector.tensor_copy(rhs2[:, :, 0:1], mv2[:, :, 0:1])
        nc.vector.tensor_mul(rhs2[:, :, 1:2], mv2[:, :, 0:1], mv2[:, :, 0:1])
        nc.vector.tensor_add(rhs2[:, :, 1:2], rhs2[:, :, 1:2], mv2[:, :, 1:2])

        psg = ps_acc.tile([GPT, CT * 2], F32, tag="acc", name="psg")
        nc.tensor.matmul(
            psg, gind_sb, rhs2.rearrange("p a b -> p (a b)"),
            start=True, stop=True,
        )
        gst = gnst.tile([GPT, CT * 2], F32, tag="gst")
        nc.vector.tensor_scalar_mul(gst, psg, 1.0 / GS)
        pscb = ps_acc.tile([P, CT * 2], F32, tag="acc", name="pscb")
        nc.tensor.matmul(pscb, gindT_sb, gst, start=True, stop=True)
        cb_t = gnst.tile([P, CT, 2], F32, tag="cb")
        nc.vector.tensor_copy(cb_t.rearrange("p a b -> p (a b)"), pscb)

        varb = gnst.tile([P, CT], F32, tag="varb")
        nc.vector.tensor_mul(varb, cb_t[:, :, 0], cb_t[:, :, 0])
        nc.vector.tensor_sub(varb, cb_t[:, :, 1], varb)
        sd = gnst.tile([P, CT], F32, tag="sd")
        nc.scalar.activation(sd, varb, AF.Sqrt, bias=eps_sb)
        # load the Exp table now - nothing else touches ACT tables after.
        # Reading sd (not eps) forces the scheduler to keep this AFTER the
        # real SQRT; otherwise it hoists the preload and pays 2 extra
        # table switches.
        scratch2 = const.tile([P, 1], F32, tag="scr2")
        nc.scalar.activation(scratch2, sd[:, 0:1], AF.Exp, bias=nl4_sb)
        rstd = gnst.tile([P, CT], F32, tag="rstd")
        nc.vector.reciprocal(rstd, sd)
        A2 = gnst.tile([P, CT], F32, tag="A2", name="A2")
        nc.vector.tensor_mul(A2, rstd, gamma2)
        A16 = gnst.tile([P, CT], F32, tag="A16", name="A16")
        nc.vector.tensor_scalar_mul(A16, A2, 16.0)
        MA = gnst.tile([P, CT], F32, tag="MA")
        nc.vector.tensor_mul(MA, cb_t[:, :, 0], A2)
        B2 = gnst.tile([P, CT], F32, tag="B2", name="B2")
        nc.vector.tensor_sub(B2, beta2, MA)

        xn_res = [
            persist.tile([P, TM], F32, tag=f"xnres{ct}", name=f"xnres{ct}")
            for ct in range(CT)
        ]

        # fold the group-norm affine into fp8 DoubleRow weights, with each
        # weight's bias eviction right behind its scaling on the DVE FIFO
        # (k first: its production gates the first score matmul)
        W8q = persist.tile([P, CT, C], FP8, tag="w8q")
        W8k = persist.tile([P, CT, C], FP8, tag="w8k")
        bq216 = const.tile([P, CT], F32, tag="bq216", name="bq216")
        bk216 = const.tile([P, CT], F32, tag="bk216", name="bk216")
        bv2 = const.tile([P, CT], F32, tag="bv2", name="bv2")

        def bias_job(raws, outt, bcol, scale, co):
            psb = ps_acc.tile([P, 1], F32, tag="acc", name="psb")
            for ci in range(CT):
                nc.tensor.matmul(
                    psb[:, 0:1], raws[ci][:, ts(co, P)],
                    B2[:, ci : ci + 1],
                    start=(ci == 0), stop=(ci == CT - 1),
                )
            nc.vector.tensor_scalar(
                outt[:, co : co + 1], psb[:, 0:1], bcol[:, co : co + 1],
                scale, op0=ALU.add, op1=ALU.mult,
            )

        for dst, raws, outt, bcol, scale in (
            (W8k, Wk_raw, bk216, bk_c, 16.0),
            (W8q, Wq_raw, bq216, bq_c, 16.0),
        ):
            for ci in range(CT):
                nc.vector.tensor_scalar(
                    dst[:, ci, :], raws[ci], A16[:, ci : ci + 1], None,
                    op0=ALU.mult,
                )
            for co in range(CT):
                bias_job(raws, outt, bcol, scale, co)

        # ---- q/k production (fp8 DoubleRow); evictions on DVE/GpSimd ----
        qT2 = persist.tile([P, CT, TM], FP8, tag="qT2")
        kT2 = persist.tile([P, CT, T], FP8, tag="kT2")

        def evict_ps(eng, dst, src, bias_col):
            # ACT Identity(+bias) is tableless, so the idle pre-stream ACT
            # can absorb the first evictions and shorten the DVE tail
            if eng is nc.scalar:
                nc.scalar.activation(dst, src, AF.Identity, bias=bias_col)
            else:
                eng.tensor_scalar(dst, src, bias_col, None, op0=ALU.add)

        def k_half(pr, co, h, eng):
            # [P,512] acc-bank psum so the big ring stays free for the
            # score stream (its slots would otherwise gate the first exp)
            psk = ps_acc.tile([P, 512], F32, tag="acc", name="psk")
            nc.tensor.matmul(
                psk, W8k[:, :, ts(co, P)],
                x8[:, :, ts(2 * pr + h, 512)],
                start=True, stop=True, perf_mode=DR,
            )
            evict_ps(eng, kT2[:, co, ts(2 * pr + h, 512)], psk,
                     bk216[:, co : co + 1])

        def k_job(pr, co, eng):
            for h in range(2):
                k_half(pr, co, h, eng)

        def q_job(ch, co, eng):
            psq = ps_acc.tile([P, 512], F32, tag="acc", name="psq")
            nc.tensor.matmul(
                psq, W8q[:, :, ts(co, P)], x8[:, :, ts(ch, 512)],
                start=True, stop=True, perf_mode=DR,
            )
            evict_ps(eng, qT2[:, co, ts(ch, 512)], psq,
                     bq216[:, co : co + 1])

        # q chunk 0 first, then k in score-consumption order (pair 0 = both
        # co halves of h0 first) so the DVE eviction order matches what the
        # score stream needs next. All evictions on DVE (GpSimd cannot
        # read PSUM).
        q_job(0, 0, nc.scalar)
        q_job(0, 1, nc.scalar)
        k_half(0, 0, 0, nc.scalar)
        k_half(0, 1, 0, nc.scalar)
        k_half(0, 0, 1, nc.scalar)
        k_half(0, 1, 1, nc.scalar)
        for pr in range(1, 4):
            k_job(pr, 0, nc.vector)
            k_job(pr, 1, nc.vector)
        for ch in range(1, 4):
            q_job(ch, 0, nc.vector)
            q_job(ch, 1, nc.vector)

        # bv2 (feeds fc2 only) after the k/q production
        for co in range(CT):
            bias_job(Wv_raw, bv2, bv_c, 1.0, co)

        # ---- fused projection weight Wvp = C1 @ Wp (bf16), via PE
        # transposes of C1; scheduled behind q/k, needed only by tc0's proj
        # Wp bf16 copies on GpSimd (idle here; on DVE the scheduler hoists
        # them into the gn chain where they head-of-line block on the DMA)
        Wp_sb = []
        for ci in range(CT):
            t = persist.tile([P, C], BF16, tag=f"wp{ci}", name=f"wp{ci}")
            nc.gpsimd.tensor_copy(t, Wp_raw[ci])
            Wp_sb.append(t)
        C1 = [
            persist.tile([P, C], F32, tag=f"c1_{ci}", name=f"c1_{ci}")
            for ci in range(CT)
        ]
        for ci in range(CT):
            nc.vector.tensor_scalar(
                C1[ci], Wv_raw[ci], A2[:, ci : ci + 1], None, op0=ALU.mult
            )
        C1T = persist.tile([P, CT, C], BF16, tag="c1t")  # [c-part(ch), ci]
        for ci in range(CT):
            for ch in range(CT):
                ptr = ps_acc.tile([P, P], F32, tag="acc", name="c1tr")
                nc.tensor.transpose(ptr, C1[ci][:, ts(ch, P)], ident)
                nc.vector.tensor_copy(C1T[:, ch, ts(ci, P)], ptr)
        Wvp_sb = []
        for ci in range(CT):
            psw = ps_acc.tile([P, C], F32, tag="acc", name="psw")
            for ch in range(CT):
                nc.tensor.matmul(
                    psw, C1T[:, ch, ts(ci, P)], Wp_sb[ch],
                    start=(ch == 0), stop=(ch == CT - 1),
                )
            t = persist.tile([P, C], BF16, tag=f"wvp{ci}", name=f"wvp{ci}")
            nc.vector.tensor_copy(t, psw)
            Wvp_sb.append(t)

        # fc2 = bv2 @ Wp + bp (needed by the first projection)
        fc2 = const.tile([P, CT], F32, tag="fc2")
        for co in range(CT):
            psf = ps_acc.tile([P, 1], F32, tag="acc", name=f"fc{co}p")
            for ci in range(CT):
                nc.tensor.matmul(
                    psf, Wp_raw[ci][:, ts(co, P)], bv2[:, ci : ci + 1],
                    start=(ci == 0), stop=(ci == CT - 1),
                )
            nc.vector.tensor_add(fc2[:, co : co + 1], psf, bp_c[:, co : co + 1])

        # residual xn in fp32 on gpsimd (consumed late, by the output evicts)
        for ct in range(CT):
            for ib in range(TM // 512):
                nc.gpsimd.tensor_scalar(
                    xn_res[ct][:, ts(ib, 512)], xT_sb[ct][:, ts(ib, 512)],
                    A2[:, ct : ct + 1], B2[:, ct : ct + 1],
                    op0=ALU.mult, op1=ALU.add,
                )

        # ---- attention ----
        at_p = ctx.enter_context(tc.tile_pool(name="at", bufs=12))
        oa_p = ctx.enter_context(tc.tile_pool(name="oa", bufs=2))
        fin_p = ctx.enter_context(tc.tile_pool(name="fin", bufs=2))

        def po_mm(po, ats, pair):
            # attn@[x|1] accumulation for one si pair
            for j in range(JT):
                nc.tensor.matmul(
                    po[j][:, 0 : C + 1],
                    ats[pair][:, :, ts(j, P)],
                    x8n[:, 2 * pair : 2 * pair + 2, 0 : C + 1],
                    start=(pair == 0), stop=(pair == NPAIR - 1), perf_mode=DR,
                )

        def rt_oa(tci, po, pe_transpose):
            # normalize on eviction: oa = po * (1/den), bf16; transpose to
            # [c, t] for the projection (DMA xbar, or PE on the final chunk)
            rt = fin_p.tile([P, JT], F32, tag="rt")
            oaT = [
                oa_p.tile([P, Tc], BF16, tag=f"oat{ci}", name=f"oat{ci}")
                for ci in range(CT)
            ]
            for j in range(JT):
                nc.vector.reciprocal(rt[:, j : j + 1], po[j][:, C : C + 1])
                oa_j = oa_p.tile([P, C], BF16, tag="oa", bufs=4, name="oa_j")
                nc.vector.tensor_scalar(
                    oa_j, po[j][:, 0:C], rt[:, j : j + 1], None, op0=ALU.mult
                )
                for ci in range(CT):
                    if pe_transpose:
                        ptr = ps_acc.tile([P, P], BF16, tag="acc", name="ptr")
                        nc.tensor.transpose(ptr, oa_j[:, ts(ci, P)], ident_bf)
                        nc.vector.tensor_copy(oaT[ci][:, ts(j, P)], ptr)
                    else:
                        nc.sync.dma_start(
                            oaT[ci][:, ts(j, P)], oa_j[:, ts(ci, P)],
                            transpose=True,
                        )
            return oaT

        def proj_phase(tci, oaT, use_big=False):
            # projT[co] = sum_ci Wvp[ci,co]^T @ oaT[ci]  (bf16), then
            # out^T = projT + fc2 + xn_res  (fp32 residual path)
            t0 = tci * Tc
            for co in range(CT):
                if use_big:
                    pp = ps_big.tile([P, 1024], F32, tag="big", name="pp")[:, 0:Tc]
                else:
                    pp = ps_acc.tile([P, Tc], F32, tag="acc", name="pp")
                for ci in range(CT):
                    nc.tensor.matmul(
                        pp, Wvp_sb[ci][:, ts(co, P)], oaT[ci],
                        start=(ci == 0), stop=(ci == CT - 1),
                    )
                obT = fin_p.tile([P, Tc], F32, tag="obT")
                nc.vector.scalar_tensor_tensor(
                    obT, pp, fc2[:, co : co + 1], xn_res[co][:, t0 : t0 + Tc],
                    op0=ALU.add, op1=ALU.add,
                )
                for hh in range(2):
                    eng = nc.gpsimd if (co + hh) % 2 == 0 else nc.sync
                    eng.dma_start(
                        out_d[ts(co, P), t0 + hh * 256 : t0 + (hh + 1) * 256],
                        obT[:, ts(hh, 256)],
                    )

        # si-pair loop with the previous chunk's drain (last po pairs,
        # normalize, projection) pipelined into this chunk's slack so the
        # exp stream never sees a long chunk boundary.
        drain = None        # (tci, po, ats, lag) with `lag` pairs outstanding
        projq = None        # (tci, oaT) awaiting projection
        LAG0 = 8            # tc0: po banks inherit from late-evicted prologue
        for tci in range(NT):
            lag = LAG0 if tci == 0 else 5
            qrhs = qT2[:, :, tci * Tc : (tci + 1) * Tc]
            po = None
            ats = []
            for p in range(NPAIR):
                pss2 = ps_big.tile([P, 1024], F32, tag="big", name="pss2")
                for i in range(2):
                    nc.tensor.matmul(
                        pss2[:, ts(i, 512)],
                        kT2[:, :, ts(2 * p + i, P)],
                        qrhs,
                        start=True, stop=True, perf_mode=DR,
                    )
                if drain is not None and p <= drain[3] - 1:
                    po_mm(drain[1], drain[2], NPAIR - drain[3] + p)
                    if p == drain[3] - 1:
                        oaT_prev = rt_oa(drain[0], drain[1], False)
                        if projq is not None:
                            proj_phase(*projq)
                        projq = (drain[0], oaT_prev)
                        drain = None
                at2 = at_p.tile([P, 2, Tc], FP8, tag="at")
                nc.scalar.activation(
                    at2.rearrange("p a b -> p (a b)"), pss2, AF.Exp,
                    scale=sc16, bias=nl4_sb,
                )
                ats.append(at2)
                # lag-5 po: on tc0 the accumulators inherit psum banks from
                # late-evicted prologue psums; the extra lag (plus the deep
                # at pool) rides out that slot wait without stalling the
                # score/exp stream
                if p == lag:
                    po = [
                        ps_acc.tile([P, VC], F32, tag="acc", name=f"po{j}")
                        for j in range(JT)
                    ]
                if p >= lag:
                    po_mm(po, ats, p - lag)
            drain = (tci, po, ats, lag)

        # flush: the tc2 projection first (its PE matmuls overlap the drain's
        # psum evictions on DVE), then the last chunk per-j pipelined:
        # po tail -> normalize -> PE transpose -> project -> evict -> DMA,
        # so the output DMAs start as early as possible
        if projq is not None:
            proj_phase(projq[0], projq[1], use_big=True)
            projq = None
        po, ats = drain[1], drain[2]
        for pair in range(NPAIR - 5, NPAIR - 1):
            po_mm(po, ats, pair)
        t0 = drain[0] * Tc
        rt = fin_p.tile([P, JT], F32, tag="rt")
        oaT = [
            oa_p.tile([P, Tc], BF16, tag=f"oat{ci}", name=f"oat{ci}")
            for ci in range(CT)
        ]
        pp = [
            ps_big.tile([P, 1024], F32, tag="big", name=f"ppf{co}")[:, 0:Tc]
            for co in range(CT)
        ]
        obT = [
            fin_p.tile([P, Tc], F32, tag=f"obf{co}", name=f"obf{co}")
            for co in range(CT)
        ]
        for j in range(JT):
            # last po pair for this j only, then drain j immediately
            nc.tensor.matmul(
                po[j][:, 0 : C + 1],
                ats[NPAIR - 1][:, :, ts(j, P)],
                x8n[:, 2 * (NPAIR - 1) : 2 * NPAIR, 0 : C + 1],
                start=False, stop=True, perf_mode=DR,
            )
            nc.vector.reciprocal(rt[:, j : j + 1], po[j][:, C : C + 1])
            oa_j = oa_p.tile([P, C], BF16, tag="oa", bufs=4, name="oa_j")
            nc.vector.tensor_scalar(
                oa_j, po[j][:, 0:C], rt[:, j : j + 1], None, op0=ALU.mult
            )
            for ci in range(CT):
                ptr = ps_acc.tile([P, P], BF16, tag="acc", name="ptr")
                nc.tensor.transpose(ptr, oa_j[:, ts(ci, P)], ident_bf)
                nc.vector.tensor_copy(oaT[ci][:, ts(j, P)], ptr)
            for co in range(CT):
                for ci in range(CT):
                    nc.tensor.matmul(
                        pp[co][:, ts(j, P)], Wvp_sb[ci][:, ts(co, P)],
                        oaT[ci][:, ts(j, P)],
                        start=(ci == 0), stop=(ci == CT - 1),
                    )
                nc.vector.scalar_tensor_tensor(
                    obT[co][:, ts(j, P)], pp[co][:, ts(j, P)],
                    fc2[:, co : co + 1], xn_res[co][:, t0 + j * P : t0 + (j + 1) * P],
                    op0=ALU.add, op1=ALU.add,
                )
                eng = nc.gpsimd if (co + j) % 2 == 0 else nc.sync
                eng.dma_start(
                    out_d[ts(co, P), t0 + j * P : t0 + (j + 1) * P],
                    obT[co][:, ts(j, P)],
                )

    _legalize_waits(nc)
    return nc


# Embedded sync-wait capacity per BIR opcode in walrus codegen. A matmul
# lowers to an S3_LW struct with a single wait slot; DMA direct2d carries two.
# Excess waits are hoisted onto standalone EventSemaphore instructions placed
# immediately before the owner on the same engine queue.
_WAIT_BUDGET = {"Matmult": 1}
_DEFAULT_BUDGET = 1
_NO_BUDGET = {"EventSemaphore", "AllEngineBarrier", "SemaphoreOp"}
_MAX_EV_WAITS = 1


def _legalize_waits(nc):
    n = 0
    for fn in nc.m.functions:
        for blk in fn.blocks:
            insts = blk.instructions
            out = []
            changed = False
            for inst in insts:
                if inst.opcode in _NO_BUDGET:
                    out.append(inst)
                    continue
                budget = _WAIT_BUDGET.get(inst.opcode, _DEFAULT_BUDGET)
                si = inst.sync_info
                waits = list(si.on_wait or []) if si is not None else []
                if len(waits) > budget:
                    extra, keep = waits[:-budget], waits[-budget:]
                    while extra:
                        chunk, extra = extra[:_MAX_EV_WAITS], extra[_MAX_EV_WAITS:]
                        ev = mybir.InstEventSemaphore(
                            name=f"{inst.name}-wsplit{n}",
                            engine=inst.engine,
                            ins=[],
                            outs=[],
                            sync_info=mybir.SyncInfo(on_wait=chunk, on_update=[]),
                        )
                        n += 1
                        nc.register_instruction(ev, overwrite=True)
                        out.append(ev)
                    si.on_wait = keep
                    inst.sync_info = si
                    changed = True
                out.append(inst)
            if changed:
                blk.instructions = out
    return nc


_NC_CACHE = {}


def _get_nc(T=4096, C=256):
    key = (T, C)
    if key not in _NC_CACHE:
        _NC_CACHE[key] = build_nc(T=T, C=C)
    return _NC_CACHE[key]


def make_in_maps(x, gamma, beta, Wq, bq, Wk, bk, Wv, bv, Wp, bp):
    import ml_dtypes

    B, H, W, C = x.shape
    T = H * W
    TM = T // 2
    NS = T // P
    VC = 272
    GS = C // GROUPS
    FP8NP = ml_dtypes.float8_e4m3

    xf = np.ascontiguousarray(np.asarray(x, np.float32).reshape(B, T, C))
    gind = np.zeros((P, P // GS), np.float32)
    for p in range(P):
        gind[p, p // GS] = 1.0
    gindT = np.ascontiguousarray(gind.T)

    common = {
        "gamma": np.asarray(gamma, np.float32),
        "beta": np.asarray(beta, np.float32),
        "Wq": np.asarray(Wq, np.float32),
        "Wk": np.asarray(Wk, np.float32),
        "Wv": np.asarray(Wv, np.float32),
        "Wp": np.asarray(Wp, np.float32),
        "bq": np.asarray(bq, np.float32),
        "bk": np.asarray(bk, np.float32),
        "bv": np.asarray(bv, np.float32),
        "bp": np.asarray(bp, np.float32),
        "gind": gind,
        "gindT": gindT,
    }

    in_maps = []
    for core in range(N_CORES):
        b, h = divmod(core, 2)
        xr = xf[b] if h == 0 else np.roll(xf[b], -TM, axis=0)
        xr8 = xr.astype(FP8NP)
        # packed natural-layout fp8 with the softmax-den ones column baked
        # in: xN8[p, st*VC + c] = fp8(xr[st*128 + p, c]), col C = 1.0
        xn8 = np.zeros((P, NS, VC), FP8NP)
        xn8[:, :, 0:C] = xr8.reshape(NS, P, C).transpose(1, 0, 2)
        xn8[:, :, C] = FP8NP(1.0)
        in_maps.append(
            {
                "xT": np.ascontiguousarray(xr.T.astype(ml_dtypes.bfloat16)),
                "xT8": np.ascontiguousarray(xr8.T),
                "xN8": np.ascontiguousarray(xn8.reshape(P, NS * VC)),
                **common,
            }
        )
    return in_maps


def gather_out(results, B, T, C):
    TM = T // 2
    out = np.empty((B, T, C), np.float32)
    for core in range(N_CORES):
        b, h = divmod(core, 2)
        out[b, h * TM : (h + 1) * TM] = results[core]["outT"].T
    return out


def kernel(x, gamma, beta, Wq, bq, Wk, bk, Wv, bv, Wp, bp):
    B, H, W, C = x.shape
    T = H * W
    nc = _get_nc(T=T, C=C)
    in_maps = make_in_maps(x, gamma, beta, Wq, bq, Wk, bk, Wv, bv, Wp, bp)
    res = run_bass_kernel_spmd(nc, in_maps, core_ids=list(range(N_CORES)))
    return gather_out(res.results, B, T, C).reshape(B, H, W, C)


# revision 64
# speedup vs baseline: 1.0471x; 1.0471x over previous
"""Trainium2 Bass kernel for an AttentionBlock:
GroupNorm(8 groups) -> q/k/v dense -> softmax(q k^T / sqrt(d)) v -> proj -> +residual(xn).

Sharding: 8 cores = (batch b in 0..3) x (half h in 0..1). Core (b, h) receives
x[b] in both layouts ([C, T] for q/k, [T, C] for attn@x) with its half of the
T=4096 tokens rolled to the front; it computes group norm + k for all tokens
and attention / projection / residual for its own 2048 query rows. Output is
produced transposed ([C, TM]); the host transposes back while gathering.

v elimination: attn_n@V@Wp == (attn_n@x) @ (diag(A_gn) Wv Wp) + 1⊗cvec
(rowsums of normalized attn are 1), so the v dense never runs: the po matmul
consumes fp8 x directly (natural [s, c] layout with a ones column for the
softmax denominator) and the projection uses the on-device fused weight
Wvp = (A_gn ⊙rows Wv) @ Wp.  cvec = (B_gn@Wv + bv)@Wp + bp is the same fc2
constant as the classic path.

Numerics: groupnorm+residual fp32; q/k/scores/exp/po in fp8-e4m3 DoubleRow
(q/k weights carry 16x for fp8 range, undone in the exp scale; exp has a
-ln(32) shift); projection bf16.

Schedule: ACT does ONLY a few tableless Copy casts, the sqrt, and the 64-exp
softmax stream (all other psum evictions live on DVE/GpSimd), the PE is
HAM-warmed with real bf16 matmuls (transposes don't count as PE-busy), x
streams in over four DMA rings, and all of q/k is produced before the stream
so the exp run is gapless.
"""

import numpy as np
from contextlib import ExitStack

import concourse.bass as bass
import concourse.tile as tile
from concourse import mybir
from concourse.bass import ts
from concourse.masks import make_identity
from concourse.bass_utils import run_bass_kernel_spmd

F32 = mybir.dt.float32
BF16 = mybir.dt.bfloat16
FP8 = mybir.dt.float8e4
AF = mybir.ActivationFunctionType
ALU = mybir.AluOpType
DR = mybir.MatmulPerfMode.DoubleRow

N_CORES = 8
GROUPS = 8
EPS = 1e-3
P = 128
LN4 = 3.4657359027997265  # ln(32): softmax-invariant shift; keeps exp < 240
N_WARM = 56               # f32 N=128 warmup matmuls: each lowers to TWO
                          # HW passes (fp32 hi/lo), ~280ns per call; they
                          # must END by stats-done or psg queues behind them


def build_nc(T=4096, C=256):
    TM = T // 2          # rows (queries) this core owns
    CT = C // P          # channel tiles (2)
    NS = T // P          # key tiles (32)
    Tc = 512             # query chunk
    NT = TM // Tc        # t-chunks of the query rows (4)
    JT = Tc // P         # 128-row output subtiles per t-chunk (4)
    NPAIR = NS // 2      # score pairs (1024-wide exp groups) per t-chunk (16)
    GS = C // GROUPS     # channels per group (32)
    GPT = P // GS        # groups per channel tile (4)
    NB = T // 512        # x chunks per channel tile (8)
    VC = 272             # x8n row stride (C + den col + pad to 16B)
    sc16 = float(C) ** -0.5 / 256.0

    assert CT == 2 and TM % Tc == 0 and T % 512 == 0

    nc = bass.Bass()

    xT_d = nc.dram_tensor("xT", [C, T], BF16, kind="ExternalInput")
    xT8_d = nc.dram_tensor("xT8", [C, T], FP8, kind="ExternalInput")
    xN8_d = nc.dram_tensor("xN8", [P, (T // P) * VC], FP8, kind="ExternalInput")
    gamma_d = nc.dram_tensor("gamma", [C], F32, kind="ExternalInput")
    beta_d = nc.dram_tensor("beta", [C], F32, kind="ExternalInput")
    Wq_d = nc.dram_tensor("Wq", [C, C], F32, kind="ExternalInput")
    Wk_d = nc.dram_tensor("Wk", [C, C], F32, kind="ExternalInput")
    Wv_d = nc.dram_tensor("Wv", [C, C], F32, kind="ExternalInput")
    Wp_d = nc.dram_tensor("Wp", [C, C], F32, kind="ExternalInput")
    bq_d = nc.dram_tensor("bq", [C], F32, kind="ExternalInput")
    bk_d = nc.dram_tensor("bk", [C], F32, kind="ExternalInput")
    bv_d = nc.dram_tensor("bv", [C], F32, kind="ExternalInput")
    bp_d = nc.dram_tensor("bp", [C], F32, kind="ExternalInput")
    gind_d = nc.dram_tensor("gind", [P, GPT], F32, kind="ExternalInput")
    gindT_d = nc.dram_tensor("gindT", [GPT, P], F32, kind="ExternalInput")
    out_d = nc.dram_tensor("outT", [C, TM], F32, kind="ExternalOutput")

    with ExitStack() as ctx:
        tc = ctx.enter_context(tile.TileContext(nc))

        const = ctx.enter_context(tc.tile_pool(name="const", bufs=1))
        persist = ctx.enter_context(tc.tile_pool(name="persist", bufs=1))
        # PSUM: acc tag = 1-bank slots x4; big tag = 2-bank slots x2 (8 banks)
        ps_acc = ctx.enter_context(tc.tile_pool(name="ps_acc", bufs=4, space="PSUM"))
        ps_big = ctx.enter_context(tc.tile_pool(name="ps_big", bufs=2, space="PSUM"))

        # ---- identity + warmup FIRST: the iota must precede the DMA
        # descriptor instructions on the gpsimd queue, or the warm matmuls
        # (and psg behind them on the PE FIFO) slip by ~7us ----
        ident = const.tile([P, P], F32, tag="ident")
        make_identity(nc, ident)
        # Real matmuls (NOT transposes - those don't count as PE-busy for
        # HAM) keep the PE warming from t~7us until the gn-stat matmuls.
        # They must depend ONLY on make_identity: any DVE-produced operand
        # gets scheduled behind the bn_stats queue.
        warm = ps_acc.tile([P, P], F32, tag="acc", name="warm")
        for _ in range(N_WARM):
            nc.tensor.matmul(warm, ident, ident, start=True, stop=True)
        ident_bf = const.tile([P, P], BF16, tag="identb")
        nc.vector.tensor_copy(ident_bf, ident)

        # ---- x loads: three rings, phase-ordered per ring so the global
        # arrival order is x-bf16 -> Wk/Wq -> xT8 halves -> Wv/Wp + xN8
        # quarters (each gated only by how soon its consumer runs) ----
        xin = ctx.enter_context(tc.tile_pool(name="xin", bufs=1))
        gnst = ctx.enter_context(tc.tile_pool(name="gnst", bufs=2))
        # x-bf16 as four 512KB transfers (big transfers spread across all
        # 16 SDMA slots); the h0 halves lead on both rings so the ib-major
        # bn_stats stream starts as soon as both h0s land
        xT_sb = []
        stats = []
        for ct in range(CT):
            xt = xin.tile([P, T], BF16, tag=f"x{ct}", name=f"x{ct}")
            st = gnst.tile([P, NB, 6], F32, tag=f"bn{ct}", name=f"bn{ct}")
            xT_sb.append(xt)
            stats.append(st)
        # ct-alternating so BOTH channel tiles of each token range land
        # together (bn_stats consumes ib-major across cts)
        for cb in range(4):
            for k2, eng in ((0, nc.sync), (1, nc.gpsimd)):
                ct = (cb + k2) % 2
                eng.dma_start(
                    xT_sb[ct][:, ts(cb, 1024)],
                    xT_d[ts(ct, P), ts(cb, 1024)],
                )

        # ACT table preloads: Sqrt now; Exp is loaded right after the real
        # SQRT so the set resident during the stream is Exp, no mid-switch.
        eps_sb = const.tile([P, 1], F32, tag="eps")
        nc.vector.memset(eps_sb, EPS)
        nl4_sb = const.tile([P, 1], F32, tag="nl4")
        nc.vector.memset(nl4_sb, -LN4)
        scratch1 = const.tile([P, 1], F32, tag="scr1")
        nc.scalar.activation(scratch1, eps_sb, AF.Sqrt, bias=eps_sb)

        # ---- constants / small parameter loads (scalar ring tail) ----
        gind_sb = const.tile([P, GPT], F32, tag="gind")
        nc.scalar.dma_start(gind_sb, gind_d[:, :])
        gindT_sb = const.tile([GPT, P], F32, tag="gindT")
        nc.scalar.dma_start(gindT_sb, gindT_d[:, :])

        def col2(dram_vec, tag):
            t = const.tile([P, CT], F32, tag=tag, name=tag)
            nc.scalar.dma_start(t, dram_vec.rearrange("(c p) -> p c", p=P))
            return t

        gamma2 = col2(gamma_d, "gamma2")
        beta2 = col2(beta_d, "beta2")
        bq_c = col2(bq_d, "bqc")
        bk_c = col2(bk_d, "bkc")
        bv_c = col2(bv_d, "bvc")
        bp_c = col2(bp_d, "bpc")

        # weight raw staging behind each ring's x-bf16 chunks: Wk on sync,
        # Wq on gpsimd (both gate the gn->k chain), Wv/Wp on scalar (late
        # consumers)
        wraw = ctx.enter_context(tc.tile_pool(name="wraw", bufs=8))

        def w_raw_tiles(dram_w, tag, eng):
            tiles = []
            for ci in range(CT):
                raw = wraw.tile([P, C], F32, tag="wraw", name=f"{tag}{ci}raw")
                eng.dma_start(raw, dram_w[ts(ci, P), :])
                tiles.append(raw)
            return tiles

        Wk_raw = w_raw_tiles(Wk_d, "wk", nc.sync)
        Wq_raw = w_raw_tiles(Wq_d, "wq", nc.gpsimd)
        Wv_raw = w_raw_tiles(Wv_d, "wv", nc.scalar)
        Wp_raw = w_raw_tiles(Wp_d, "wp", nc.scalar)
        # (smalls + Wv/Wp on the scalar ring total ~0.57MB - they trickle
        # alongside the x loads without moving the critical arrivals much)

        # fp8 copies of x come PRE-CAST from the host (xT8 [C,T]; xN8 packed
        # [P, NS*VC] natural-layout with the ones column baked in) - both
        # fully contiguous per partition, issued AFTER the weights so the
        # critical loads win the SDMA round-robin.
        x8 = persist.tile([P, CT, T], FP8, tag="x8")
        x8n = persist.tile([P, NS, VC], FP8, tag="x8n")
        for hh, eng in ((0, nc.sync), (1, nc.gpsimd)):
            eng.dma_start(
                x8[:, :, ts(hh, T // 2)],
                xT8_d[:, ts(hh, T // 2)].rearrange("(a p) t -> p a t", p=P),
            )
        x8n_flat = x8n.rearrange("p a b -> p (a b)")
        for qq, eng in ((0, nc.sync), (1, nc.gpsimd), (2, nc.sync), (3, nc.gpsimd)):
            eng.dma_start(
                x8n_flat[:, ts(qq, 8 * VC)],
                xN8_d[:, ts(qq, 8 * VC)],
            )

        # ---- fp8 casts of x^T (a few tableless Copy casts on ACT, rest on
        # DVE/GpSimd) + bn_stats on DVE, streaming behind the x DMAs ----
        for ib in range(NB):
            for ct in range(CT):
                nc.vector.bn_stats(stats[ct][:, ib, :], xT_sb[ct][:, ts(ib, 512)])

        # ---- group norm stats -> per-channel A (scale), B (shift) ----
        mv2 = gnst.tile([P, CT, 2], F32, tag="mv2")
        for ct in range(CT):
            nc.vector.bn_aggr(mv2[:, ct, :], stats[ct])
        rhs2 = gnst.tile([P, CT, 2], F32, tag="rhs2")
        nc.vector.tensor_copy(rhs2[:, :, 0:1], mv2[:, :, 0:1])
        nc.vector.tensor_mul(rhs2[:, :, 1:2], mv2[:, :, 0:1], mv2[:, :, 0:1])
        nc.vector.tensor_add(rhs2[:, :, 1:2], rhs2[:, :, 1:2], mv2[:, :, 1:2])

        psg = ps_acc.tile([GPT, CT * 2], F32, tag="acc", name="psg")
        nc.tensor.matmul(
            psg, gind_sb, rhs2.rearrange("p a b -> p (a b)"),
            start=True, stop=True,
        )
        gst = gnst.tile([GPT, CT * 2], F32, tag="gst")
        nc.vector.tensor_scalar_mul(gst, psg, 1.0 / GS)
        pscb = ps_acc.tile([P, CT * 2], F32, tag="acc", name="pscb")
        nc.tensor.matmul(pscb, gindT_sb, gst, start=True, stop=True)
        cb_t = gnst.tile([P, CT, 2], F32, tag="cb")
        nc.vector.tensor_copy(cb_t.rearrange("p a b -> p (a b)"), pscb)

        varb = gnst.tile([P, CT], F32, tag="varb")
        nc.vector.tensor_mul(varb, cb_t[:, :, 0], cb_t[:, :, 0])
        nc.vector.tensor_sub(varb, cb_t[:, :, 1], varb)
        sd = gnst.tile([P, CT], F32, tag="sd")
        nc.scalar.activation(sd, varb, AF.Sqrt, bias=eps_sb)
        # load the Exp table now - nothing else touches ACT tables after.
        # Reading sd (not eps) forces the scheduler to keep this AFTER the
        # real SQRT; otherwise it hoists the preload and pays 2 extra
        # table switches.
        scratch2 = const.tile([P, 1], F32, tag="scr2")
        nc.scalar.activation(scratch2, sd[:, 0:1], AF.Exp, bias=nl4_sb)
        rstd = gnst.tile([P, CT], F32, tag="rstd")
        nc.vector.reciprocal(rstd, sd)
        A2 = gnst.tile([P, CT], F32, tag="A2", name="A2")
        nc.vector.tensor_mul(A2, rstd, gamma2)
        A16 = gnst.tile([P, CT], F32, tag="A16", name="A16")
        nc.vector.tensor_scalar_mul(A16, A2, 16.0)
        MA = gnst.tile([P, CT], F32, tag="MA")
        nc.vector.tensor_mul(MA, cb_t[:, :, 0], A2)
        B2 = gnst.tile([P, CT], F32, tag="B2", name="B2")
        nc.vector.tensor_sub(B2, beta2, MA)

        xn_res = [
            persist.tile([P, TM], F32, tag=f"xnres{ct}", name=f"xnres{ct}")
            for ct in range(CT)
        ]

        # fold the group-norm affine into fp8 DoubleRow weights, with each
        # weight's bias eviction right behind its scaling on the DVE FIFO
        # (k first: its production gates the first score matmul)
        W8q = persist.tile([P, CT, C], FP8, tag="w8q")
        W8k = persist.tile([P, CT, C], FP8, tag="w8k")
        bq216 = const.tile([P, CT], F32, tag="bq216", name="bq216")
        bk216 = const.tile([P, CT], F32, tag="bk216", name="bk216")
        bv2 = const.tile([P, CT], F32, tag="bv2", name="bv2")

        def bias_job(raws, outt, bcol, scale, co):
            psb = ps_acc.tile([P, 1], F32, tag="acc", name="psb")
            for ci in range(CT):
                nc.tensor.matmul(
                    psb[:, 0:1], raws[ci][:, ts(co, P)],
                    B2[:, ci : ci + 1],
                    start=(ci == 0), stop=(ci == CT - 1),
                )
            nc.vector.tensor_scalar(
                outt[:, co : co + 1], psb[:, 0:1], bcol[:, co : co + 1],
                scale, op0=ALU.add, op1=ALU.mult,
            )

        for dst, raws, outt, bcol, scale in (
            (W8k, Wk_raw, bk216, bk_c, 16.0),
            (W8q, Wq_raw, bq216, bq_c, 16.0),
        ):
            for ci in range(CT):
                nc.vector.tensor_scalar(
                    dst[:, ci, :], raws[ci], A16[:, ci : ci + 1], None,
                    op0=ALU.mult,
                )
            for co in range(CT):
                bias_job(raws, outt, bcol, scale, co)

        # ---- q/k production (fp8 DoubleRow); evictions on DVE/GpSimd ----
        qT2 = persist.tile([P, CT, TM], FP8, tag="qT2")
        kT2 = persist.tile([P, CT, T], FP8, tag="kT2")

        def evict_ps(eng, dst, src, bias_col):
            # ACT Identity(+bias) is tableless, so the idle pre-stream ACT
            # can absorb the first evictions and shorten the DVE tail
            if eng is nc.scalar:
                nc.scalar.activation(dst, src, AF.Identity, bias=bias_col)
            else:
                eng.tensor_scalar(dst, src, bias_col, None, op0=ALU.add)

        def k_half(pr, co, h, eng):
            # [P,512] acc-bank psum so the big ring stays free for the
            # score stream (its slots would otherwise gate the first exp)
            psk = ps_acc.tile([P, 512], F32, tag="acc", name="psk")
            nc.tensor.matmul(
                psk, W8k[:, :, ts(co, P)],
                x8[:, :, ts(2 * pr + h, 512)],
                start=True, stop=True, perf_mode=DR,
            )
            evict_ps(eng, kT2[:, co, ts(2 * pr + h, 512)], psk,
                     bk216[:, co : co + 1])

        def k_job(pr, co, eng):
            for h in range(2):
                k_half(pr, co, h, eng)

        def q_job(ch, co, eng):
            psq = ps_acc.tile([P, 512], F32, tag="acc", name="psq")
            nc.tensor.matmul(
                psq, W8q[:, :, ts(co, P)], x8[:, :, ts(ch, 512)],
                start=True, stop=True, perf_mode=DR,
            )
            evict_ps(eng, qT2[:, co, ts(ch, 512)], psq,
                     bq216[:, co : co + 1])

        # q chunk 0 first, then k in score-consumption order (pair 0 = both
        # co halves of h0 first) so the DVE eviction order matches what the
        # score stream needs next. All evictions on DVE (GpSimd cannot
        # read PSUM).
        q_job(0, 0, nc.vector)
        q_job(0, 1, nc.vector)
        k_half(0, 0, 0, nc.scalar)
        k_half(0, 1, 0, nc.scalar)
        k_half(0, 0, 1, nc.scalar)
        k_half(0, 1, 1, nc.scalar)
        for pr in range(1, 4):
            k_job(pr, 0, nc.vector)
            k_job(pr, 1, nc.vector)
        for ch in range(1, 4):
            q_job(ch, 0, nc.vector)
            q_job(ch, 1, nc.vector)

        # bv2 (feeds fc2 only) after the k/q production
        for co in range(CT):
            bias_job(Wv_raw, bv2, bv_c, 1.0, co)

        # ---- fused projection weight Wvp = C1 @ Wp (bf16), via PE
        # transposes of C1; scheduled behind q/k, needed only by tc0's proj
        # Wp bf16 copies on GpSimd (idle here; on DVE the scheduler hoists
        # them into the gn chain where they head-of-line block on the DMA)
        Wp_sb = []
        for ci in range(CT):
            t = persist.tile([P, C], BF16, tag=f"wp{ci}", name=f"wp{ci}")
            nc.gpsimd.tensor_copy(t, Wp_raw[ci])
            Wp_sb.append(t)
        C1 = [
            persist.tile([P, C], F32, tag=f"c1_{ci}", name=f"c1_{ci}")
            for ci in range(CT)
        ]
        for ci in range(CT):
            nc.vector.tensor_scalar(
                C1[ci], Wv_raw[ci], A2[:, ci : ci + 1], None, op0=ALU.mult
            )
        C1T = persist.tile([P, CT, C], BF16, tag="c1t")  # [c-part(ch), ci]
        for ci in range(CT):
            for ch in range(CT):
                ptr = ps_acc.tile([P, P], F32, tag="acc", name="c1tr")
                nc.tensor.transpose(ptr, C1[ci][:, ts(ch, P)], ident)
                nc.vector.tensor_copy(C1T[:, ch, ts(ci, P)], ptr)
        Wvp_sb = []
        for ci in range(CT):
            psw = ps_acc.tile([P, C], F32, tag="acc", name="psw")
            for ch in range(CT):
                nc.tensor.matmul(
                    psw, C1T[:, ch, ts(ci, P)], Wp_sb[ch],
                    start=(ch == 0), stop=(ch == CT - 1),
                )
            t = persist.tile([P, C], BF16, tag=f"wvp{ci}", name=f"wvp{ci}")
            nc.vector.tensor_copy(t, psw)
            Wvp_sb.append(t)

        # fc2 = bv2 @ Wp + bp (needed by the first projection)
        fc2 = const.tile([P, CT], F32, tag="fc2")
        for co in range(CT):
            psf = ps_acc.tile([P, 1], F32, tag="acc", name=f"fc{co}p")
            for ci in range(CT):
                nc.tensor.matmul(
                    psf, Wp_raw[ci][:, ts(co, P)], bv2[:, ci : ci + 1],
                    start=(ci == 0), stop=(ci == CT - 1),
                )
            nc.vector.tensor_add(fc2[:, co : co + 1], psf, bp_c[:, co : co + 1])

        # residual xn in fp32 on gpsimd (consumed late, by the output evicts)
        for ct in range(CT):
            for ib in range(TM // 512):
                nc.gpsimd.tensor_scalar(
                    xn_res[ct][:, ts(ib, 512)], xT_sb[ct][:, ts(ib, 512)],
                    A2[:, ct : ct + 1], B2[:, ct : ct + 1],
                    op0=ALU.mult, op1=ALU.add,
                )

        # ---- attention ----
        at_p = ctx.enter_context(tc.tile_pool(name="at", bufs=12))
        oa_p = ctx.enter_context(tc.tile_pool(name="oa", bufs=2))
        fin_p = ctx.enter_context(tc.tile_pool(name="fin", bufs=2))

        def po_mm(po, ats, pair):
            # attn@[x|1] accumulation for one si pair
            for j in range(JT):
                nc.tensor.matmul(
                    po[j][:, 0 : C + 1],
                    ats[pair][:, :, ts(j, P)],
                    x8n[:, 2 * pair : 2 * pair + 2, 0 : C + 1],
                    start=(pair == 0), stop=(pair == NPAIR - 1), perf_mode=DR,
                )

        def rt_oa(tci, po, pe_transpose):
            # normalize on eviction: oa = po * (1/den), bf16; transpose to
            # [c, t] for the projection (DMA xbar, or PE on the final chunk)
            rt = fin_p.tile([P, JT], F32, tag="rt")
            oaT = [
                oa_p.tile([P, Tc], BF16, tag=f"oat{ci}", name=f"oat{ci}")
                for ci in range(CT)
            ]
            for j in range(JT):
                nc.vector.reciprocal(rt[:, j : j + 1], po[j][:, C : C + 1])
                oa_j = oa_p.tile([P, C], BF16, tag="oa", bufs=4, name="oa_j")
                nc.vector.tensor_scalar(
                    oa_j, po[j][:, 0:C], rt[:, j : j + 1], None, op0=ALU.mult
                )
                for ci in range(CT):
                    if pe_transpose:
                        ptr = ps_acc.tile([P, P], BF16, tag="acc", name="ptr")
                        nc.tensor.transpose(ptr, oa_j[:, ts(ci, P)], ident_bf)
                        nc.vector.tensor_copy(oaT[ci][:, ts(j, P)], ptr)
                    else:
                        nc.sync.dma_start(
                            oaT[ci][:, ts(j, P)], oa_j[:, ts(ci, P)],
                            transpose=True,
                        )
            return oaT

        def proj_phase(tci, oaT, use_big=False):
            # projT[co] = sum_ci Wvp[ci,co]^T @ oaT[ci]  (bf16), then
            # out^T = projT + fc2 + xn_res  (fp32 residual path)
            t0 = tci * Tc
            for co in range(CT):
                if use_big:
                    pp = ps_big.tile([P, 1024], F32, tag="big", name="pp")[:, 0:Tc]
                else:
                    pp = ps_acc.tile([P, Tc], F32, tag="acc", name="pp")
                for ci in range(CT):
                    nc.tensor.matmul(
                        pp, Wvp_sb[ci][:, ts(co, P)], oaT[ci],
                        start=(ci == 0), stop=(ci == CT - 1),
                    )
                obT = fin_p.tile([P, Tc], F32, tag="obT")
                nc.vector.scalar_tensor_tensor(
                    obT, pp, fc2[:, co : co + 1], xn_res[co][:, t0 : t0 + Tc],
                    op0=ALU.add, op1=ALU.add,
                )
                for hh in range(2):
                    eng = nc.gpsimd if (co + hh) % 2 == 0 else nc.sync
                    eng.dma_start(
                        out_d[ts(co, P), t0 + hh * 256 : t0 + (hh + 1) * 256],
                        obT[:, ts(hh, 256)],
                    )

        # si-pair loop with the previous chunk's drain (last po pairs,
        # normalize, projection) pipelined into this chunk's slack so the
        # exp stream never sees a long chunk boundary.
        drain = None        # (tci, po, ats, lag) with `lag` pairs outstanding
        projq = None        # (tci, oaT) awaiting projection
        LAG0 = 8            # tc0: po banks inherit from late-evicted prologue
        for tci in range(NT):
            lag = LAG0 if tci == 0 else 5
            qrhs = qT2[:, :, tci * Tc : (tci + 1) * Tc]
            po = None
            ats = []
            for p in range(NPAIR):
                pss2 = ps_big.tile([P, 1024], F32, tag="big", name="pss2")
                for i in range(2):
                    nc.tensor.matmul(
                        pss2[:, ts(i, 512)],
                        kT2[:, :, ts(2 * p + i, P)],
                        qrhs,
                        start=True, stop=True, perf_mode=DR,
                    )
                if drain is not None and p <= drain[3] - 1:
                    po_mm(drain[1], drain[2], NPAIR - drain[3] + p)
                    if p == drain[3] - 1:
                        oaT_prev = rt_oa(drain[0], drain[1], False)
                        if projq is not None:
                            proj_phase(*projq)
                        projq = (drain[0], oaT_prev)
                        drain = None
                at2 = at_p.tile([P, 2, Tc], FP8, tag="at")
                nc.scalar.activation(
                    at2.rearrange("p a b -> p (a b)"), pss2, AF.Exp,
                    scale=sc16, bias=nl4_sb,
                )
                ats.append(at2)
                # lag-5 po: on tc0 the accumulators inherit psum banks from
                # late-evicted prologue psums; the extra lag (plus the deep
                # at pool) rides out that slot wait without stalling the
                # score/exp stream
                if p == lag:
                    po = [
                        ps_acc.tile([P, VC], F32, tag="acc", name=f"po{j}")
                        for j in range(JT)
                    ]
                if p >= lag:
                    po_mm(po, ats, p - lag)
            drain = (tci, po, ats, lag)

        # flush: the tc2 projection first (its PE matmuls overlap the drain's
        # psum evictions on DVE), then the last chunk per-j pipelined:
        # po tail -> normalize -> PE transpose -> project -> evict -> DMA,
        # so the output DMAs start as early as possible
        if projq is not None:
            proj_phase(projq[0], projq[1], use_big=True)
            projq = None
        po, ats = drain[1], drain[2]
        for pair in range(NPAIR - 5, NPAIR - 1):
            po_mm(po, ats, pair)
        t0 = drain[0] * Tc
        rt = fin_p.tile([P, JT], F32, tag="rt")
        oaT = [
            oa_p.tile([P, Tc], BF16, tag=f"oat{ci}", name=f"oat{ci}")
            for ci in range(CT)
        ]
        pp = [
            ps_big.tile([P, 1024], F32, tag="big", name=f"ppf{co}")[:, 0:Tc]
            for co in range(CT)
        ]
        obT = [
            fin_p.tile([P, Tc], F32, tag=f"obf{co}", name=f"obf{co}")
            for co in range(CT)
        ]
        for j in range(JT):
            # last po pair for this j only, then drain j immediately
            nc.tensor.matmul(
                po[j][:, 0 : C + 1],
                ats[NPAIR - 1][:, :, ts(j, P)],
                x8n[:, 2 * (NPAIR - 1) : 2 * NPAIR, 0 : C + 1],
                start=False, stop=True, perf_mode=DR,
            )
            nc.vector.reciprocal(rt[:, j : j + 1], po[j][:, C : C + 1])
            oa_j = oa_p.tile([P, C], BF16, tag="oa", bufs=4, name="oa_j")
            nc.vector.tensor_scalar(
                oa_j, po[j][:, 0:C], rt[:, j : j + 1], None, op0=ALU.mult
            )
            for ci in range(CT):
                ptr = ps_acc.tile([P, P], BF16, tag="acc", name="ptr")
                nc.tensor.transpose(ptr, oa_j[:, ts(ci, P)], ident_bf)
                nc.vector.tensor_copy(oaT[ci][:, ts(j, P)], ptr)
            for co in range(CT):
                for ci in range(CT):
                    nc.tensor.matmul(
                        pp[co][:, ts(j, P)], Wvp_sb[ci][:, ts(co, P)],
                        oaT[ci][:, ts(j, P)],
                        start=(ci == 0), stop=(ci == CT - 1),
                    )
                nc.vector.scalar_tensor_tensor(
                    obT[co][:, ts(j, P)], pp[co][:, ts(j, P)],
                    fc2[:, co : co + 1], xn_res[co][:, t0 + j * P : t0 + (j + 1) * P],
                    op0=ALU.add, op1=ALU.add,
                )
                eng = nc.gpsimd if (co + j) % 2 == 0 else nc.sync
                eng.dma_start(
                    out_d[ts(co, P), t0 + j * P : t0 + (j + 1) * P],
                    obT[co][:, ts(j, P)],
                )

    _legalize_waits(nc)
    return nc


# Embedded sync-wait capacity per BIR opcode in walrus codegen. A matmul
# lowers to an S3_LW struct with a single wait slot; DMA direct2d carries two.
# Excess waits are hoisted onto standalone EventSemaphore instructions placed
# immediately before the owner on the same engine queue.
_WAIT_BUDGET = {"Matmult": 1}
_DEFAULT_BUDGET = 1
_NO_BUDGET = {"EventSemaphore", "AllEngineBarrier", "SemaphoreOp"}
_MAX_EV_WAITS = 1


def _legalize_waits(nc):
    n = 0
    for fn in nc.m.functions:
        for blk in fn.blocks:
            insts = blk.instructions
            out = []
            changed = False
            for inst in insts:
                if inst.opcode in _NO_BUDGET:
                    out.append(inst)
                    continue
                budget = _WAIT_BUDGET.get(inst.opcode, _DEFAULT_BUDGET)
                si = inst.sync_info
                waits = list(si.on_wait or []) if si is not None else []
                if len(waits) > budget:
                    extra, keep = waits[:-budget], waits[-budget:]
                    while extra:
                        chunk, extra = extra[:_MAX_EV_WAITS], extra[_MAX_EV_WAITS:]
                        ev = mybir.InstEventSemaphore(
                            name=f"{inst.name}-wsplit{n}",
                            engine=inst.engine,
                            ins=[],
                            outs=[],
                            sync_info=mybir.SyncInfo(on_wait=chunk, on_update=[]),
                        )
                        n += 1
                        nc.register_instruction(ev, overwrite=True)
                        out.append(ev)
                    si.on_wait = keep
                    inst.sync_info = si
                    changed = True
                out.append(inst)
            if changed:
                blk.instructions = out
    return nc


_NC_CACHE = {}


def _get_nc(T=4096, C=256):
    key = (T, C)
    if key not in _NC_CACHE:
        _NC_CACHE[key] = build_nc(T=T, C=C)
    return _NC_CACHE[key]


def make_in_maps(x, gamma, beta, Wq, bq, Wk, bk, Wv, bv, Wp, bp):
    import ml_dtypes

    B, H, W, C = x.shape
    T = H * W
    TM = T // 2
    NS = T // P
    VC = 272
    GS = C // GROUPS
    FP8NP = ml_dtypes.float8_e4m3

    xf = np.ascontiguousarray(np.asarray(x, np.float32).reshape(B, T, C))
    gind = np.zeros((P, P // GS), np.float32)
    for p in range(P):
        gind[p, p // GS] = 1.0
    gindT = np.ascontiguousarray(gind.T)

    common = {
        "gamma": np.asarray(gamma, np.float32),
        "beta": np.asarray(beta, np.float32),
        "Wq": np.asarray(Wq, np.float32),
        "Wk": np.asarray(Wk, np.float32),
        "Wv": np.asarray(Wv, np.float32),
        "Wp": np.asarray(Wp, np.float32),
        "bq": np.asarray(bq, np.float32),
        "bk": np.asarray(bk, np.float32),
        "bv": np.asarray(bv, np.float32),
        "bp": np.asarray(bp, np.float32),
        "gind": gind,
        "gindT": gindT,
    }

    in_maps = []
    for core in range(N_CORES):
        b, h = divmod(core, 2)
        xr = xf[b] if h == 0 else np.roll(xf[b], -TM, axis=0)
        xr8 = xr.astype(FP8NP)
        # packed natural-layout fp8 with the softmax-den ones column baked
        # in: xN8[p, st*VC + c] = fp8(xr[st*128 + p, c]), col C = 1.0
        xn8 = np.zeros((P, NS, VC), FP8NP)
        xn8[:, :, 0:C] = xr8.reshape(NS, P, C).transpose(1, 0, 2)
        xn8[:, :, C] = FP8NP(1.0)
        in_maps.append(
            {
                "xT": np.ascontiguousarray(xr.T.astype(ml_dtypes.bfloat16)),
                "xT8": np.ascontiguousarray(xr8.T),
                "xN8": np.ascontiguousarray(xn8.reshape(P, NS * VC)),
                **common,
            }
        )
    return in_maps


def gather_out(results, B, T, C):
    TM = T // 2
    out = np.empty((B, T, C), np.float32)
    for core in range(N_CORES):
        b, h = divmod(core, 2)
        out[b, h * TM : (h + 1) * TM] = results[core]["outT"].T
    return out


def kernel(x, gamma, beta, Wq, bq, Wk, bk, Wv, bv, Wp, bp):
    B, H, W, C = x.shape
    T = H * W
    nc = _get_nc(T=T, C=C)
    in_maps = make_in_maps(x, gamma, beta, Wq, bq, Wk, bk, Wv, bv, Wp, bp)
    res = run_bass_kernel_spmd(nc, in_maps, core_ids=list(range(N_CORES)))
    return gather_out(res.results, B, T, C).reshape(B, H, W, C)
